# revision 2
# baseline (speedup 1.0000x reference)
"""Trainium2 Bass kernel for nn_CSTR: B=4096-lane vmapped 2047-step rollout.

Strategy: pure data-parallel over 8 cores (512 lanes each). Slot-major packed
layout: SBUF partition p = 32*s + g holds slot s (component) of lane-group g;
free dim = 16 lanes. Cross-component affine maps run on the TensorEngine as
block-structured [128,128] matmuls (kron(A.T, I32)); tanh/sigmoid/square on
ScalarE; elementwise products/blends on VectorE/GpSimd. The three shifted RK4
tanh stages are Taylor-collapsed (exact to ~1e-9 per step):
  (tanh(a) + 4*tanh(a+H/2) + tanh(a+H))*(H/6) = H*t + 5e-5*(1-t^2) - 3.3e-7*t(1-t^2)
with the cubic term dropped (validated: max rel err 7e-6 on J).
Stage cost accumulates in PSUM across all steps via start=False matmuls.
"""
import os
import sys
import numpy as np
from contextlib import ExitStack

sys.path.insert(0, "/opt/trn_rl_repo")

import concourse.bacc as bacc
import concourse.bass as bass
import concourse.mybir as mybir
import concourse.tile as tile
from concourse import bass_utils

F32 = mybir.dt.float32
AF = mybir.ActivationFunctionType

# problem constants
H = np.float32(0.01)
LAM = np.float32(1.0)
B_TOT, N_CORES = 4096, 8
LANES = B_TOT // N_CORES          # 512
NG, NL = 32, 16                   # 32 groups * 16 free lanes
T_REAL = 2047

# fp32 derived constants
C1 = np.float32(1.0) - H            # 0.99
GC = np.float32(0.5) * H            # 0.005
EC = np.float32(H * H / 2)          # 5e-5
A0 = np.float32(5e-5)
ALPHA = np.float32(H - np.float32(1e-6) / 3)


def _kron32(A):
    """lhsT for slot-space map A[out_slot, in_slot] -> [32*in, 32*out]."""
    return np.kron(np.ascontiguousarray(A.T), np.eye(NG, dtype=np.float32)).astype(np.float32)


def _build_weights(K, L, M, Mo):
    K1, K2 = np.float32(K[0, 0]), np.float32(K[0, 1])
    Ls = ((L + L.T) * np.float32(0.5)).astype(np.float32)
    Mv = M[0].astype(np.float32)
    # G slots: s0=fp2(->xh2'), s1=fp1(->xh1'), s2=x2', s3=x1'
    # rx order in reference: (x1', x2', fp1, fp2)
    sig = [3, 2, 1, 0]   # G-slot -> rx index
    W = {}
    A_R = np.diag([C1, C1, C1, C1]).astype(np.float32)
    W["W_R"] = _kron32(A_R)
    # T slots: (t4, t3, t2, t1) = tanh of R slots (xh2, xh1, x2, x1)
    A_T = np.zeros((4, 4), np.float32)
    A_T[0, 1] = -H      # fp2 <- -H*t3
    A_T[1, 0] = ALPHA   # fp1 <- alpha*t4
    A_T[2, 3] = -H      # x2' <- -H*t1
    A_T[3, 2] = ALPHA   # x1' <- alpha*t2
    W["W_T"] = _kron32(A_T)
    # Q slots: (q4, q3, q2, q1)
    A_Q = np.zeros((4, 4), np.float32)
    A_Q[1, 0] = -A0     # fp1 <- -a0*q4
    A_Q[3, 2] = -A0     # x1' <- -a0*q2
    W["W_Q"] = _kron32(A_Q)
    # AUX slots: (u, 1)
    A_X = np.zeros((4, 2), np.float32)
    A_X[0] = [GC, -EC]   # fp2
    A_X[1] = [H, A0]     # fp1
    A_X[2] = [GC, -EC]   # x2'
    A_X[3] = [H, A0]     # x1'
    W["W_AUX"] = _kron32(A_X)
    # phi quadratic: V[s] = sum_s' Ls[sig s, sig s'] G[s']
    A_V = np.zeros((4, 4), np.float32)
    for s in range(4):
        for sp in range(4):
            A_V[s, sp] = Ls[sig[s], sig[sp]]
    W["W_V"] = _kron32(A_V)
    W["W_P1"] = _kron32(np.ones((2, 4), np.float32))
    A_PM = np.zeros((2, 4), np.float32)
    for s in range(4):
        A_PM[0, s] = Mv[sig[s]]
        A_PM[1, s] = Mv[sig[s]]
    W["W_PM"] = _kron32(A_PM)
    A_DIF = np.array([[-1, 0, 1, 0], [0, -1, 0, 1]], np.float32)
    W["W_DIF"] = _kron32(A_DIF)
    A_K = np.array([[K2, K1, 0, 0]], np.float32)   # u' = K1*xh1' + K2*xh2'
    W["W_K"] = _kron32(A_K)
    # cost quadratic Qc in rx space over (x1', x2')
    Qc = np.zeros((4, 4), np.float32)
    Qc[0, 0] = 1 + K1 * K1
    Qc[0, 1] = Qc[1, 0] = K1 * K2
    Qc[1, 1] = 1 + K2 * K2
    A_V2 = np.zeros((4, 4), np.float32)
    for s in range(4):
        for sp in range(4):
            A_V2[s, sp] = Qc[sig[s], sig[sp]]
    W["W_V2"] = _kron32(A_V2)
    W["W_A1"] = _kron32(np.ones((1, 4), np.float32))
    W["W_AD"] = _kron32(np.array([[LAM, 0]], np.float32))
    A_I = np.array([[0, np.float32(1.0) + K1 * K1 + LAM]], np.float32)
    W["W_INIT"] = _kron32(A_I)
    W["W_FIN"] = _kron32(np.array([[0, 0, 10, 10]], np.float32))
    return W, float(Mo[0, 0]), float(K1)


def _build_program(T2, NQ, BLK, STATIC_TAIL, Mo_f, K1_f, t_real, t2_loop=None):
    """Emit the bass program. Returns nc."""
    QS = T2 // NQ                 # steps per w-quarter tile
    if t2_loop is None:
        t2_loop = T2
    nc = bacc.Bacc("TRN2", target_bir_lowering=False, debug=False)

    wd = [nc.dram_tensor(f"wq{q}", [128, QS * NL], F32, kind="ExternalInput")
          for q in range(NQ)]
    wnames = ["W_R", "W_T", "W_Q", "W_AUX", "W_V", "W_P1", "W_PM", "W_DIF",
              "W_K", "W_V2", "W_A1", "W_AD", "W_INIT", "W_FIN"]
    wshapes = {"W_R": (128, 128), "W_T": (128, 128), "W_Q": (128, 128),
               "W_AUX": (64, 128), "W_V": (128, 128), "W_P1": (128, 64),
               "W_PM": (128, 64), "W_DIF": (128, 64), "W_K": (128, 32),
               "W_V2": (128, 128), "W_A1": (128, 32), "W_AD": (64, 32),
               "W_INIT": (64, 32), "W_FIN": (128, 32)}
    wdram = {n: nc.dram_tensor(n, list(wshapes[n]), F32, kind="ExternalInput")
             for n in wnames}
    out_d = nc.dram_tensor("out", [32, NL], F32, kind="ExternalOutput")

    ctx = ExitStack()
    with tile.TileContext(nc) as tc:
        with tc.tile_pool(name="wq", bufs=1) as wpool, \
             tc.tile_pool(name="wt", bufs=1) as cpool, \
             tc.tile_pool(name="sb", bufs=3) as sp, \
             tc.tile_pool(name="ps", bufs=1, space="PSUM") as pp, \
             tc.tile_pool(name="pacc", bufs=1, space="PSUM") as pa:

            # --- load weights + w quarters ---
            wt = {}
            for n in wnames:
                wt[n] = cpool.tile(list(wshapes[n]), F32, tag=n, name=n+"_t")
                nc.sync.dma_start(wt[n][:, :], wdram[n].ap())
            wq = []
            for q in range(NQ):
                t = wpool.tile([128, QS * NL], F32, tag=f"wq{q}", name=f"wq{q}_t")
                nc.sync.dma_start(t[:, :], wd[q].ap())
                wq.append(t)

            # --- init state ---
            # R slots: (x2, x1, xh2, xh1) = (0, 1, 0, 1)
            # Ping-pong state buffers; BLK must be even so each For_i body
            # ends with state back in GA (loop-closed).
            GA = sp.tile([128, NL], F32, tag="GA", bufs=1, name="GA_t")
            GB = sp.tile([128, NL], F32, tag="GB", bufs=1, name="GB_t")
            R0 = GA
            nc.vector.memset(R0[0:32, :], 0.0)
            nc.vector.memset(R0[32:64, :], 1.0)
            nc.vector.memset(R0[64:96, :], 0.0)
            nc.vector.memset(R0[96:128, :], 1.0)
            AUX = sp.tile([64, NL], F32, tag="AUX")
            nc.vector.memset(AUX[0:32, :], K1_f)
            nc.vector.memset(AUX[32:64, :], 1.0)
            MOB = cpool.tile([64, 1], F32, tag="MOB", name="MOB_t")
            nc.vector.memset(MOB[:, :], Mo_f)

            ACC = pa.tile([32, NL], F32, tag="ACC")
            nc.tensor.matmul(ACC[:, :], wt["W_INIT"][:, :], AUX[:, :],
                             start=True, stop=False)

            state = {"R": R0, "cur": 0}

            def emit_step(wview, do_cost=True, do_dyn=True):
                R = state["R"]
                Gout = GB if state["cur"] == 0 else GA
                Tt = sp.tile([128, NL], F32, tag="T")
                nc.scalar.activation(Tt[:, :], R[:, :], AF.Tanh)
                Qt = sp.tile([128, NL], F32, tag="Qt")
                nc.scalar.activation(Qt[:, :], Tt[:, :], AF.Square)
                Fp = pp.tile([128, NL], F32, tag="F")
                nc.tensor.matmul(Fp[:, :], wt["W_R"][:, :], R[:, :],
                                 start=True, stop=False)
                nc.tensor.matmul(Fp[:, :], wt["W_T"][:, :], Tt[:, :],
                                 start=False, stop=False)
                nc.tensor.matmul(Fp[:, :], wt["W_Q"][:, :], Qt[:, :],
                                 start=False, stop=False)
                nc.tensor.matmul(Fp[:, :], wt["W_AUX"][:, :], AUX[:, :],
                                 start=False, stop=True)
                G = Gout
                # G = F + w  (w padded tile: slots (w2, w1, 0, 0))
                nc.vector.tensor_add(G[:, :], Fp[:, :], wview)
                if do_dyn:
                    Vp = pp.tile([128, NL], F32, tag="V")
                    nc.tensor.matmul(Vp[:, :], wt["W_V"][:, :], G[:, :],
                                     start=True, stop=True)
                    P = sp.tile([128, NL], F32, tag="P")
                    nc.vector.tensor_mul(P[:, :], G[:, :], Vp[:, :])
                    PH = pp.tile([64, NL], F32, tag="PHI")
                    nc.tensor.matmul(PH[:, :], wt["W_P1"][:, :], P[:, :],
                                     start=True, stop=False)
                    nc.tensor.matmul(PH[:, :], wt["W_PM"][:, :], G[:, :],
                                     start=False, stop=True)
                    D = sp.tile([64, NL], F32, tag="D")
                    nc.scalar.activation(D[:, :], PH[:, :], AF.Sigmoid,
                                         bias=MOB[:, :])
                    DIF = pp.tile([64, NL], F32, tag="DIF")
                    nc.tensor.matmul(DIF[:, :], wt["W_DIF"][:, :], G[:, :],
                                     start=True, stop=True)
                    M1 = sp.tile([64, NL], F32, tag="M1")
                    nc.vector.tensor_mul(M1[:, :], D[:, :], DIF[:, :])
                    nc.gpsimd.tensor_add(G[0:64, :], G[0:64, :], M1[:, :])
                    Up = pp.tile([32, NL], F32, tag="U")
                    nc.tensor.matmul(Up[:, :], wt["W_K"][:, :], G[:, :],
                                     start=True, stop=True)
                    nc.scalar.activation(AUX[0:32, :], Up[:, :], AF.Copy)
                    if do_cost:
                        V2 = pp.tile([128, NL], F32, tag="V2")
                        nc.tensor.matmul(V2[:, :], wt["W_V2"][:, :],
                                         G[:, :], start=True, stop=True)
                        P2 = sp.tile([128, NL], F32, tag="P2")
                        nc.vector.tensor_mul(P2[:, :], G[:, :], V2[:, :])
                        nc.tensor.matmul(ACC[:, :], wt["W_A1"][:, :],
                                         P2[:, :], start=False, stop=False)
                        nc.tensor.matmul(ACC[:, :], wt["W_AD"][:, :],
                                         D[:, :], start=False, stop=False)
                state["R"] = G
                state["cur"] ^= 1

            # --- main loops ---
            n_loop = t2_loop - STATIC_TAIL      # steps covered by For_i loops
            step_global = 0
            for q in range(NQ):
                q_lo, q_hi = q * QS, (q + 1) * QS
                lo = max(q_lo, step_global)
                dyn_hi = min(q_hi, n_loop)
                n_dyn = dyn_hi - lo
                if n_dyn > 0:
                    nblk = n_dyn // BLK
                    assert nblk * BLK == n_dyn
                    if nblk > 1:
                        with tc.For_i(0, nblk, 1) as iv:
                            for j in range(BLK):
                                off = (lo - q_lo) * NL
                                wv = wq[q][:, bass.ds(iv * (BLK * NL) + off + j * NL, NL)]
                                emit_step(wv)
                    else:
                        for j in range(BLK):
                            wv = wq[q][:, (lo - q_lo + j) * NL:(lo - q_lo + j + 1) * NL]
                            emit_step(wv)
                    step_global = dyn_hi
                # static portion inside this quarter
                for k in range(max(step_global, q_lo), min(q_hi, t2_loop)):
                    if k >= t_real:
                        break
                    j = k - q_lo
                    wv = wq[q][:, j * NL:(j + 1) * NL]
                    emit_step(wv, do_cost=(k <= t_real - 3),
                              do_dyn=(k <= t_real - 2))
                    step_global = k + 1

            # --- epilogue: final cost 10*(x1^2 + x2^2) ---
            G = state["R"]
            SQ = sp.tile([128, NL], F32, tag="P2")
            nc.vector.tensor_mul(SQ[:, :], G[:, :], G[:, :])
            nc.tensor.matmul(ACC[:, :], wt["W_FIN"][:, :], SQ[:, :],
                             start=False, stop=True)
            OUT = sp.tile([32, NL], F32, tag="OUT")
            nc.scalar.activation(OUT[:, :], ACC[:, :], AF.Copy)
            nc.sync.dma_start(out_d.ap(), OUT[:, :])
    ctx.close()
    nc.compile()
    return nc


def _pack_w(w_core, T2, NQ):
    """w_core [512, 2, T_real] -> list of NQ arrays [128, (T2/NQ)*16].

    Padded tile layout per step: slots (0, 0, w2, w1)."""
    QS = T2 // NQ
    T = w_core.shape[2]
    arr = np.zeros((T2, 4, NG, NL), np.float32)
    wc = w_core.reshape(NG, NL, 2, T)
    arr[:T, 2] = np.moveaxis(wc[:, :, 1, :], -1, 0)   # w2
    arr[:T, 3] = np.moveaxis(wc[:, :, 0, :], -1, 0)   # w1
    # [T2, 4, NG, NL] -> quarters [QS, 128, NL] -> [128, QS*NL]
    out = []
    for q in range(NQ):
        a = arr[q * QS:(q + 1) * QS].reshape(QS, 128, NL)
        out.append(np.ascontiguousarray(a.transpose(1, 0, 2)).reshape(128, QS * NL))
    return out


_PROG_CACHE = {}


def kernel(w, K, L, M, Mo):
    w = np.asarray(w, np.float32)
    K = np.asarray(K, np.float32)
    L = np.asarray(L, np.float32)
    M = np.asarray(M, np.float32)
    Mo = np.asarray(Mo, np.float32)
    B = w.shape[0]
    T2, NQ, BLK, STATIC_TAIL = 2048, 4, 64, 64
    Wmats, Mo_f, K1_f = _build_weights(K, L, M, Mo)

    key = (w.shape, K.tobytes(), L.tobytes(), M.tobytes(), Mo.tobytes())
    if key not in _PROG_CACHE:
        _PROG_CACHE[key] = _build_program(T2, NQ, BLK, STATIC_TAIL, Mo_f,
                                          K1_f, T_REAL)
    nc = _PROG_CACHE[key]

    in_maps = []
    for c in range(N_CORES):
        m = {n: Wmats[n] for n in Wmats}
        wqs = _pack_w(w[c * LANES:(c + 1) * LANES], T2, NQ)
        for q in range(NQ):
            m[f"wq{q}"] = wqs[q]
        in_maps.append(m)

    kw = {}
    if os.environ.get("KERNEL_TRACE"):
        kw = dict(trace=True)
        if os.environ.get("KERNEL_TRACE_DIR"):
            kw["tmpdir"] = os.environ["KERNEL_TRACE_DIR"]
    res = bass_utils.run_bass_kernel_spmd(nc, in_maps,
                                          core_ids=list(range(N_CORES)), **kw)
    globals()["_LAST_RES"] = res
    out = np.empty(B, np.float32)
    for c in range(N_CORES):
        o = res.results[c]["out"]          # [32, 16]
        out[c * LANES:(c + 1) * LANES] = o.reshape(LANES)
    return out



# revision 8
# speedup vs baseline: 1.3764x; 1.3764x over previous
"""Trainium2 Bass kernel for nn_CSTR: B=4096-lane vmapped 2047-step rollout.

v2: data-parallel over 8 cores (512 lanes each), 2 independent 256-lane
chains per core interleaved op-by-op to hide cross-engine latency.

Per chain: slot-major layout, slot = 8 partitions x 32 free lanes.
State slots (xh1, xh2, x1, x2). Mega input tile X[120,32] bf16 =
[state@0; tanh@32; tanh^2@64; xdup@96; ones@112] (all engine-written
sub-views 32-partition aligned). The state update collapses to ONE bf16
matmul producing the O(H) increment Delta (u = K@xhat composed in, RK4
constants on the ones slot); exact fp32 state G[48,32] (state+xdup) is
kept via a vector add G' = Delta + (G + w). Gating: W_VD ->
[Ls@rx + M; x-fp; pad; Qc@x], products on DVE, W_PH -> phi (x4 dup),
sigmoid on ScalarE, blend on DVE/GpSimd. Stage cost accumulates into a
persistent PSUM bank every 4 steps from two [128,32] staging tiles
(sigmoid deltas + Qc products). Only O(H)-scaled terms flow through
bf16 matmuls; fp32 trajectory exactness is preserved via vector adds
(validated 2.7e-4 max rel err vs reference in numpy).
"""
import os
import sys
import numpy as np
import ml_dtypes
from contextlib import ExitStack
from itertools import zip_longest

sys.path.insert(0, "/opt/trn_rl_repo")

import concourse.bacc as bacc
import concourse.bass as bass
import concourse.mybir as mybir
import concourse.tile as tile
from concourse import bass_utils

F32 = mybir.dt.float32
BF16 = mybir.dt.bfloat16
AF = mybir.ActivationFunctionType

H = np.float32(0.01)
LAM = np.float32(1.0)
B_TOT, N_CORES = 4096, 8
LPC = 512                 # lanes per core
NCH = 2                   # chains per core
LCH = 256                 # lanes per chain
NG, NL = 8, 32            # groups x free lanes per chain
TR = 2047                 # real steps
CH = 256                  # steps per w chunk
NQ = 8                    # chunks (last holds 255 real steps)

C1 = np.float32(1.0) - H
GC = np.float32(0.5) * H
EC = np.float32(H * H / 2)
A0 = np.float32(5e-5)
ALPHA = np.float32(H - np.float32(1e-6) / 3)


def _kron8(A):
    """lhsT for slot map A[out_slot, in_slot] -> [8*in, 8*out] bf16."""
    k = np.kron(np.ascontiguousarray(A.T), np.eye(NG, dtype=np.float32))
    return k.astype(ml_dtypes.bfloat16)


def _build_weights(K, L, M, Mo):
    K1, K2 = np.float32(K[0, 0]), np.float32(K[0, 1])
    Ls = ((L + L.T) * np.float32(0.5)).astype(np.float32)
    Mv = M[0].astype(np.float32)
    Qc = np.array([[1 + K1 * K1, K1 * K2], [K1 * K2, 1 + K2 * K2]], np.float32)
    sig = [2, 3, 0, 1]   # slot -> rx index; rx = (x1, x2, fp1, fp2)

    # input slot order: state(4) T(4) Q(4) xdup(2) ones(1) = 15 slots
    # W_X out: [Delta(4); Delta-xdup(2)]
    uc = np.array([H, GC, H, GC], np.float32)
    Kv = np.array([K1, K2, 0, 0], np.float32)
    A_S = (C1 - 1) * np.eye(4, dtype=np.float32) + np.outer(uc, Kv)
    A_T = np.zeros((4, 4), np.float32)
    A_T[0, 1] = ALPHA; A_T[1, 0] = -H; A_T[2, 3] = ALPHA; A_T[3, 2] = -H
    A_Q = np.zeros((4, 4), np.float32)
    A_Q[0, 1] = -A0; A_Q[2, 3] = -A0
    A_ONES = np.array([A0, -EC, A0, -EC], np.float32).reshape(4, 1)
    WX4 = np.concatenate(
        [A_S, A_T, A_Q, np.zeros((4, 2), np.float32), A_ONES], axis=1)
    WX = np.concatenate([WX4, WX4[2:4]], axis=0)          # [6, 15]

    # W_VD: same 15 input slots -> [y(4); DIF(2); pad(2); y2(2)] = 10 out
    Avd = np.zeros((10, 15), np.float32)
    for s in range(4):
        for sp in range(4):
            Avd[s, sp] = Ls[sig[s], sig[sp]]
        Avd[s, 14] = Mv[sig[s]]
    Avd[4, 2] = 1; Avd[4, 0] = -1          # DIF1 = x1 - xh1
    Avd[5, 3] = 1; Avd[5, 1] = -1          # DIF2 = x2 - xh2
    Avd[8, 2] = Qc[0, 0]; Avd[8, 3] = Qc[0, 1]
    Avd[9, 2] = Qc[1, 0]; Avd[9, 3] = Qc[1, 1]

    APH = np.ones((4, 4), np.float32)      # delta x4 dup
    AC4D = np.zeros((1, 16), np.float32)
    for b in range(4):
        AC4D[0, 4 * b] = LAM               # first delta copy per block
    AC4P = np.zeros((1, 16), np.float32)
    for b in range(4):
        AC4P[0, 4 * b] = 1; AC4P[0, 4 * b + 1] = 1
    AC1D = np.zeros((1, 4), np.float32); AC1D[0, 0] = LAM
    AC1P = np.zeros((1, 4), np.float32); AC1P[0, 0] = 1; AC1P[0, 1] = 1
    AFIN = np.array([[10.0, 10.0]], np.float32)

    W = {
        "WX": _kron8(WX),        # [120, 48]
        "WVD": _kron8(Avd),      # [120, 80]
        "WPH": _kron8(APH),      # [32, 32]
        "WAC4D": _kron8(AC4D),   # [128, 8]
        "WAC4P": _kron8(AC4P),   # [128, 8]
        "WAC1D": _kron8(AC1D),   # [32, 8]
        "WAC1P": _kron8(AC1P),   # [32, 8]
        "WFIN": _kron8(AFIN),    # [16, 8]
    }
    init_c = float(1.0 + K1 * K1 + LAM)
    return W, float(Mo[0, 0]), init_c


WSHAPES = {"WX": (120, 48), "WVD": (120, 80), "WPH": (32, 32),
           "WAC4D": (128, 8), "WAC4P": (128, 8), "WAC1D": (32, 8),
           "WAC1P": (32, 8), "WFIN": (16, 8)}


def _init_consts():
    """Host constants: X init [120,32] bf16 (A: state set, B: ones only)
    and G init [48,32] f32."""
    xa = np.zeros((120, NL), np.float32)
    for lo, v in ((0, 1.0), (8, 0.0), (16, 1.0), (24, 0.0)):   # state
        xa[lo:lo + 8] = v
    xa[96:104] = 1.0; xa[104:112] = 0.0                        # xdup
    xa[112:120] = 1.0                                          # ones
    xb = np.zeros((120, NL), np.float32)
    xb[112:120] = 1.0
    gi = np.zeros((48, NL), np.float32)
    gi[0:8] = 1.0; gi[16:24] = 1.0; gi[32:40] = 1.0
    return (xa.astype(ml_dtypes.bfloat16), xb.astype(ml_dtypes.bfloat16),
            gi)


def _build_program(Mo_f, init_c):
    nc = bacc.Bacc("TRN2", target_bir_lowering=False, debug=False)

    wd = {}
    for c in range(NCH):
        for q in range(NQ):
            wd[(c, q)] = nc.dram_tensor(
                f"w{c}_{q}", [48, CH * NL], F32, kind="ExternalInput")
    wdram = {n: nc.dram_tensor(n, list(WSHAPES[n]), BF16, kind="ExternalInput")
             for n in WSHAPES}
    xa_d = nc.dram_tensor("XIA", [120, NL], BF16, kind="ExternalInput")
    xb_d = nc.dram_tensor("XIB", [120, NL], BF16, kind="ExternalInput")
    gi_d = nc.dram_tensor("GI", [48, NL], F32, kind="ExternalInput")
    out_d = [nc.dram_tensor(f"out{c}", [NG, NL], F32, kind="ExternalOutput")
             for c in range(NCH)]

    ctx = ExitStack()
    with tile.TileContext(nc) as tc:
        with tc.tile_pool(name="wq", bufs=2) as wpool, \
             tc.tile_pool(name="wt", bufs=1) as cpool, \
             tc.tile_pool(name="sb", bufs=3) as sp, \
             tc.tile_pool(name="ps", bufs=1, space="PSUM") as pp, \
             tc.tile_pool(name="pacc", bufs=1, space="PSUM") as pa:

            wt = {}
            for n in WSHAPES:
                wt[n] = cpool.tile(list(WSHAPES[n]), BF16, tag=n, name=n + "_t")
                nc.sync.dma_start(wt[n][:, :], wdram[n].ap())

            wtiles = {}

            def get_wtile(c, q):
                if (c, q) not in wtiles:
                    t = wpool.tile([48, CH * NL], F32, tag=f"w{c}",
                                   name=f"w{c}_{q}_t")
                    nc.sync.dma_start(t[:, :], wd[(c, q)].ap())
                    wtiles[(c, q)] = t
                return wtiles[(c, q)]

            chs = []
            for c in range(NCH):
                S = {}
                S["X"] = [cpool.tile([120, NL], BF16, tag=f"XA{c}", name=f"XA{c}"),
                          cpool.tile([120, NL], BF16, tag=f"XB{c}", name=f"XB{c}")]
                S["G"] = [cpool.tile([48, NL], F32, tag=f"GA{c}", name=f"GA{c}"),
                          cpool.tile([48, NL], F32, tag=f"GB{c}", name=f"GB{c}")]
                S["RW"] = cpool.tile([48, NL], F32, tag=f"RW{c}", name=f"RW{c}")
                S["SG"] = cpool.tile([128, NL], BF16, tag=f"SG{c}", name=f"SG{c}")
                S["SP"] = cpool.tile([128, NL], BF16, tag=f"SP{c}", name=f"SP{c}")
                S["MOB"] = cpool.tile([32, 1], F32, tag=f"MOB{c}", name=f"MOB{c}")
                S["ACC"] = pa.tile([8, NL], F32, tag=f"ACC{c}", name=f"ACC{c}")
                S["tagD"] = f"Dp{c}"
                S["tagV"] = f"VDp{c}"
                S["tagP"] = f"PHp{c}"
                S["tagPP"] = f"PP{c}"
                S["tagM"] = f"M1{c}"
                chs.append(S)

            Z = cpool.tile([32, NL], BF16, tag="Z", name="Z")
            nc.vector.memset(Z[:, :], 0.0)

            for c, S in enumerate(chs):
                nc.sync.dma_start(S["X"][0][:, :], xa_d.ap())
                nc.sync.dma_start(S["X"][1][:, :], xb_d.ap())
                nc.sync.dma_start(S["G"][0][:, :], gi_d.ap())
                nc.vector.memset(S["SG"][:, :], 0.0)
                nc.gpsimd.memset(S["SP"][:, :], 0.0)
                nc.vector.memset(S["MOB"][:, :], Mo_f)
                nc.tensor.matmul(S["ACC"][:, :], wt["WAC1D"][:, :], Z[:, :],
                                 start=True, stop=False)
                w0 = get_wtile(c, 0)
                nc.vector.tensor_add(S["RW"][:, :], S["G"][0][:, :],
                                     w0[:, 0:NL])
                get_wtile(c, 1)

            def step_ops(S, p, wv_next, b, do_gate, do_cost, do_acc):
                """One step of one chain; p = input parity, b = SG block."""
                Xi, Xo = S["X"][p], S["X"][1 - p]
                Go = S["G"][1 - p]
                RW, SG, SP = S["RW"], S["SG"], S["SP"]
                Dp = pp.tile([48, NL], F32, tag=S["tagD"])
                ops = []
                ops.append(lambda: nc.scalar.activation(
                    Xi[32:64, :], Xi[0:32, :], AF.Tanh))
                ops.append(lambda: nc.scalar.activation(
                    Xi[64:96, :], Xi[32:64, :], AF.Square))
                ops.append(lambda: nc.tensor.matmul(
                    Dp[:, :], wt["WX"][:, :], Xi[:, :],
                    start=True, stop=True))
                ops.append(lambda: nc.vector.tensor_add(
                    Xo[0:32, :], Dp[0:32, :], RW[0:32, :]))
                ops.append(lambda: nc.vector.tensor_add(
                    Go[:, :], Dp[:, :], RW[:, :]))
                if do_gate:
                    VDp = pp.tile([80, NL], F32, tag=S["tagV"])
                    PHp = pp.tile([32, NL], F32, tag=S["tagP"])
                    PP = sp.tile([32, NL], BF16, tag=S["tagPP"])
                    M1 = sp.tile([16, NL], F32, tag=S["tagM"])
                    ops.append(lambda: nc.vector.tensor_add(
                        Xo[96:112, :], Dp[32:48, :], RW[32:48, :]))
                    ops.append(lambda: nc.tensor.matmul(
                        VDp[:, :], wt["WVD"][:, :], Xo[:, :],
                        start=True, stop=True))
                    ops.append(lambda: nc.vector.tensor_mul(
                        PP[:, :], Xo[0:32, :], VDp[0:32, :]))
                    if do_cost:
                        ops.append(lambda: nc.vector.tensor_mul(
                            SP[32 * b:32 * b + 16, :],
                            Xo[96:112, :], VDp[64:80, :]))
                    ops.append(lambda: nc.tensor.matmul(
                        PHp[:, :], wt["WPH"][:, :], PP[:, :],
                        start=True, stop=True))
                    ops.append(lambda: nc.scalar.activation(
                        SG[32 * b:32 * b + 32, :], PHp[:, :], AF.Sigmoid,
                        bias=S["MOB"][:, :]))
                    ops.append(lambda: nc.vector.tensor_mul(
                        M1[:, :], SG[32 * b:32 * b + 16, :], VDp[32:48, :]))
                    ops.append(lambda: nc.gpsimd.tensor_add(
                        Go[0:16, :], Go[0:16, :], M1[:, :]))
                    ops.append(lambda: nc.gpsimd.tensor_add(
                        Xo[0:16, :], Xo[0:16, :], M1[:, :]))
                    if do_acc:
                        ops.append(lambda: nc.tensor.matmul(
                            S["ACC"][:, :], wt["WAC4D"][:, :], SG[:, :],
                            start=False, stop=False))
                        ops.append(lambda: nc.tensor.matmul(
                            S["ACC"][:, :], wt["WAC4P"][:, :], SP[:, :],
                            start=False, stop=False))
                if wv_next is not None:
                    ops.append(lambda: nc.gpsimd.tensor_add(
                        RW[:, :], Go[:, :], wv_next))
                return ops

            def emit_group(k0, wts, wts_next_chunk):
                """4 steps (k0..k0+3) for both chains, op-interleaved."""
                for j in range(4):
                    k = k0 + j
                    do_gate = k <= TR - 2
                    do_cost = k <= TR - 3
                    do_acc = do_cost and (j == 3)
                    opsl = []
                    for c, S in enumerate(chs):
                        wtile, base = wts[c]
                        if k + 1 <= TR - 1:
                            if j == 3 and wts_next_chunk is not None:
                                nwtile, nbase = wts_next_chunk[c]
                                wv_next = nwtile[:, nbase:nbase + NL]
                            elif isinstance(base, int):
                                nb = base + (j + 1) * NL
                                wv_next = wtile[:, nb:nb + NL]
                            else:
                                wv_next = wtile[:, bass.ds(base + (j + 1) * NL, NL)]
                        else:
                            wv_next = None
                        opsl.append(step_ops(S, j % 2, wv_next, j,
                                             do_gate, do_cost, do_acc))
                    for pair in zip_longest(*opsl):
                        for op in pair:
                            if op is not None:
                                op()

            # chunks of 256 steps: 63 hw-loop groups + 4 static tail steps
            for q in range(NQ):
                for c in range(NCH):
                    get_wtile(c, q)
                with tc.For_i(0, 63, 1) as iv:
                    wts = [(wtiles[(c, q)], iv * (4 * NL)) for c in range(NCH)]
                    emit_group(q * CH, wts, None)
                k0 = q * CH + 252
                nxt = None
                if q + 1 < NQ:
                    nxt = [(get_wtile(c, q + 1), 0) for c in range(NCH)]
                wts = [(wtiles[(c, q)], 252 * NL) for c in range(NCH)]
                emit_group(k0, wts, nxt)

            # epilogue: after 2047 steps state parity lands in G[1]
            for c, S in enumerate(chs):
                Gl = S["G"][1]
                FSQ = sp.tile([16, NL], BF16, tag=f"FSQ{c}")
                nc.vector.tensor_mul(FSQ[:, :], Gl[32:48, :], Gl[32:48, :])
                nc.tensor.matmul(S["ACC"][:, :], wt["WFIN"][:, :], FSQ[:, :],
                                 start=False, stop=False)
                nc.tensor.matmul(S["ACC"][:, :], wt["WAC1D"][:, :],
                                 S["SG"][0:32, :], start=False, stop=False)
                nc.tensor.matmul(S["ACC"][:, :], wt["WAC1P"][:, :],
                                 S["SP"][0:32, :], start=False, stop=True)
                OUT = sp.tile([8, NL], F32, tag=f"OUT{c}")
                nc.scalar.activation(OUT[:, :], S["ACC"][:, :], AF.Copy,
                                     bias=float(init_c))
                nc.sync.dma_start(out_d[c].ap(), OUT[:, :])
    ctx.close()
    nc.compile()
    return nc


def _pack_w(w_core):
    """w_core [512, 2, 2047] f32 -> {(c,q): [48, 256*32] f32}."""
    out = {}
    T2 = NQ * CH
    for c in range(NCH):
        wc = w_core[c * LCH:(c + 1) * LCH].reshape(NG, NL, 2, TR)
        arr = np.zeros((48, T2, NL), np.float32)
        for g in range(NG):
            arr[16 + g, :TR, :] = wc[g, :, 0, :].T
            arr[24 + g, :TR, :] = wc[g, :, 1, :].T
            arr[32 + g, :TR, :] = wc[g, :, 0, :].T
            arr[40 + g, :TR, :] = wc[g, :, 1, :].T
        for q in range(NQ):
            out[(c, q)] = np.ascontiguousarray(
                arr[:, q * CH:(q + 1) * CH, :]).reshape(48, CH * NL)
    return out


_PROG_CACHE = {}


def kernel(w, K, L, M, Mo):
    w = np.asarray(w, np.float32)
    K = np.asarray(K, np.float32)
    L = np.asarray(L, np.float32)
    M = np.asarray(M, np.float32)
    Mo = np.asarray(Mo, np.float32)
    B = w.shape[0]
    Wmats, Mo_f, init_c = _build_weights(K, L, M, Mo)

    key = (w.shape, K.tobytes(), L.tobytes(), M.tobytes(), Mo.tobytes())
    if key not in _PROG_CACHE:
        _PROG_CACHE[key] = _build_program(Mo_f, init_c)
    nc = _PROG_CACHE[key]

    xa, xb, gi = _init_consts()
    in_maps = []
    for core in range(N_CORES):
        m = {n: np.asarray(Wmats[n]) for n in Wmats}
        m["XIA"], m["XIB"], m["GI"] = xa, xb, gi
        wp = _pack_w(w[core * LPC:(core + 1) * LPC])
        for (c, q), arr in wp.items():
            m[f"w{c}_{q}"] = arr
        in_maps.append(m)

    kw = {}
    if os.environ.get("KERNEL_TRACE"):
        kw = dict(trace=True)
        if os.environ.get("KERNEL_TRACE_DIR"):
            kw["tmpdir"] = os.environ["KERNEL_TRACE_DIR"]
    res = bass_utils.run_bass_kernel_spmd(nc, in_maps,
                                          core_ids=list(range(N_CORES)), **kw)
    globals()["_LAST_RES"] = res
    out = np.empty(B, np.float32)
    for core in range(N_CORES):
        for c in range(NCH):
            o = res.results[core][f"out{c}"]       # [8, 32]
            lo = core * LPC + c * LCH
            out[lo:lo + LCH] = o.reshape(LCH)
    return out


# revision 12
# speedup vs baseline: 1.4948x; 1.0860x over previous
"""Trainium2 Bass kernel for nn_CSTR: B=4096-lane vmapped 2047-step rollout.

v2: data-parallel over 8 cores (512 lanes each), 2 independent 256-lane
chains per core interleaved op-by-op to hide cross-engine latency.

Per chain: slot-major layout, slot = 8 partitions x 32 free lanes.
State slots (xh1, xh2, x1, x2). Mega input tile X[120,32] bf16 =
[state@0; tanh@32; tanh^2@64; xdup@96; ones@112] (all engine-written
sub-views 32-partition aligned). The state update collapses to ONE bf16
matmul producing the O(H) increment Delta (u = K@xhat composed in, RK4
constants on the ones slot); exact fp32 state G[48,32] (state+xdup) is
kept via a vector add G' = Delta + (G + w). Gating: W_VD ->
[Ls@rx + M; x-fp; pad; Qc@x], products on DVE, W_PH -> phi (x4 dup),
sigmoid on ScalarE, blend on DVE/GpSimd. Stage cost accumulates into a
persistent PSUM bank every 4 steps from two [128,32] staging tiles
(sigmoid deltas + Qc products). Only O(H)-scaled terms flow through
bf16 matmuls; fp32 trajectory exactness is preserved via vector adds
(validated 2.7e-4 max rel err vs reference in numpy).
"""
import os
import sys
import numpy as np
import ml_dtypes
from contextlib import ExitStack
from itertools import zip_longest

sys.path.insert(0, "/opt/trn_rl_repo")

import concourse.bacc as bacc
import concourse.bass as bass
import concourse.mybir as mybir
import concourse.tile as tile
from concourse import bass_utils

F32 = mybir.dt.float32
BF16 = mybir.dt.bfloat16
AF = mybir.ActivationFunctionType



H = np.float32(0.01)
LAM = np.float32(1.0)
B_TOT, N_CORES = 4096, 8
LPC = 512                 # lanes per core
NCH = 2                   # chains per core
LCH = 256                 # lanes per chain
NG, NL = 8, 32            # groups x free lanes per chain
TR = 2047                 # real steps
CH = 256                  # steps per w chunk
NQ = 8                    # chunks (last holds 255 real steps)

C1 = np.float32(1.0) - H
GC = np.float32(0.5) * H
EC = np.float32(H * H / 2)
A0 = np.float32(5e-5)
ALPHA = np.float32(H - np.float32(1e-6) / 3)


def _kron8(A):
    """lhsT for slot map A[out_slot, in_slot] -> [8*in, 8*out] bf16."""
    k = np.kron(np.ascontiguousarray(A.T), np.eye(NG, dtype=np.float32))
    return k.astype(ml_dtypes.bfloat16)


def _build_weights(K, L, M, Mo):
    K1, K2 = np.float32(K[0, 0]), np.float32(K[0, 1])
    Ls = ((L + L.T) * np.float32(0.5)).astype(np.float32)
    Mv = M[0].astype(np.float32)
    Qc = np.array([[1 + K1 * K1, K1 * K2], [K1 * K2, 1 + K2 * K2]], np.float32)
    sig = [2, 3, 0, 1]   # slot -> rx index; rx = (x1, x2, fp1, fp2)

    # input slot order: state(4) T(4) Q(4) xdup(2) ones(1) = 15 slots
    # W_X out: [Delta(4); Delta-xdup(2)]
    uc = np.array([H, GC, H, GC], np.float32)
    Kv = np.array([K1, K2, 0, 0], np.float32)
    A_S = (C1 - 1) * np.eye(4, dtype=np.float32) + np.outer(uc, Kv)
    A_T = np.zeros((4, 4), np.float32)
    A_T[0, 1] = ALPHA; A_T[1, 0] = -H; A_T[2, 3] = ALPHA; A_T[3, 2] = -H
    A_Q = np.zeros((4, 4), np.float32)
    A_Q[0, 1] = -A0; A_Q[2, 3] = -A0
    A_ONES = np.array([A0, -EC, A0, -EC], np.float32).reshape(4, 1)
    WX4 = np.concatenate(
        [A_S, A_T, A_Q, np.zeros((4, 2), np.float32), A_ONES], axis=1)
    WX = np.concatenate([WX4, WX4[2:4]], axis=0)          # [6, 15]

    # W_VD: same 15 input slots -> [y(4); DIF(2); pad(2); y2(2)] = 10 out
    Avd = np.zeros((10, 15), np.float32)
    for s in range(4):
        for sp in range(4):
            Avd[s, sp] = Ls[sig[s], sig[sp]]
        Avd[s, 14] = Mv[sig[s]]
    Avd[4, 2] = 1; Avd[4, 0] = -1          # DIF1 = x1 - xh1
    Avd[5, 3] = 1; Avd[5, 1] = -1          # DIF2 = x2 - xh2
    Avd[8, 2] = Qc[0, 0]; Avd[8, 3] = Qc[0, 1]
    Avd[9, 2] = Qc[1, 0]; Avd[9, 3] = Qc[1, 1]

    APH = np.ones((4, 4), np.float32)      # delta x4 dup
    AC4D = np.zeros((1, 16), np.float32)
    for b in range(4):
        AC4D[0, 4 * b] = LAM               # first delta copy per block
    AC4P = np.zeros((1, 16), np.float32)
    for b in range(4):
        AC4P[0, 4 * b] = 1; AC4P[0, 4 * b + 1] = 1
    AC1D = np.zeros((1, 4), np.float32); AC1D[0, 0] = LAM
    AC1P = np.zeros((1, 4), np.float32); AC1P[0, 0] = 1; AC1P[0, 1] = 1
    AFIN = np.array([[10.0, 10.0]], np.float32)

    W = {
        "WX": _kron8(WX),        # [120, 48]
        "WVD": _kron8(Avd),      # [120, 80]
        "WPH": _kron8(APH),      # [32, 32]
        "WAC4D": _kron8(AC4D),   # [128, 8]
        "WAC4P": _kron8(AC4P),   # [128, 8]
        "WAC1D": _kron8(AC1D),   # [32, 8]
        "WAC1P": _kron8(AC1P),   # [32, 8]
        "WFIN": _kron8(AFIN),    # [16, 8]
    }
    init_c = float(1.0 + K1 * K1 + LAM)
    return W, float(Mo[0, 0]), init_c


WSHAPES = {"WX": (120, 48), "WVD": (120, 80), "WPH": (32, 32),
           "WAC4D": (128, 8), "WAC4P": (128, 8), "WAC1D": (32, 8),
           "WAC1P": (32, 8), "WFIN": (16, 8)}


def _init_consts():
    """Host constants: X init [120,32] bf16 (A: state set, B: ones only)
    and G init [48,32] f32."""
    xa = np.zeros((120, NL), np.float32)
    for lo, v in ((0, 1.0), (8, 0.0), (16, 1.0), (24, 0.0)):   # state
        xa[lo:lo + 8] = v
    xa[96:104] = 1.0; xa[104:112] = 0.0                        # xdup
    xa[112:120] = 1.0                                          # ones
    xb = np.zeros((120, NL), np.float32)
    xb[112:120] = 1.0
    gi = np.zeros((48, NL), np.float32)
    gi[0:8] = 1.0; gi[16:24] = 1.0; gi[32:40] = 1.0
    return (xa.astype(ml_dtypes.bfloat16), xb.astype(ml_dtypes.bfloat16),
            gi)


def _build_program(Mo_f, init_c):
    nc = bacc.Bacc("TRN2", target_bir_lowering=False, debug=False)

    wd = {}
    for c in range(NCH):
        for q in range(NQ):
            wd[(c, q)] = nc.dram_tensor(
                f"w{c}_{q}", [48, CH * NL], F32, kind="ExternalInput")
    wdram = {n: nc.dram_tensor(n, list(WSHAPES[n]), BF16, kind="ExternalInput")
             for n in WSHAPES}
    xa_d = nc.dram_tensor("XIA", [120, NL], BF16, kind="ExternalInput")
    xb_d = nc.dram_tensor("XIB", [120, NL], BF16, kind="ExternalInput")
    gi_d = nc.dram_tensor("GI", [48, NL], F32, kind="ExternalInput")
    out_d = [nc.dram_tensor(f"out{c}", [NG, NL], F32, kind="ExternalOutput")
             for c in range(NCH)]

    ctx = ExitStack()
    with tile.TileContext(nc) as tc:
        with tc.tile_pool(name="wq", bufs=2) as wpool, \
             tc.tile_pool(name="wt", bufs=1) as cpool, \
             tc.tile_pool(name="sb", bufs=3) as sp, \
             tc.tile_pool(name="ps", bufs=1, space="PSUM") as pp, \
             tc.tile_pool(name="pacc", bufs=1, space="PSUM") as pa:

            wt = {}
            for n in WSHAPES:
                wt[n] = cpool.tile(list(WSHAPES[n]), BF16, tag=n, name=n + "_t")
                nc.sync.dma_start(wt[n][:, :], wdram[n].ap())

            wtiles = {}

            def get_wtile(c, q):
                if (c, q) not in wtiles:
                    t = wpool.tile([48, CH * NL], F32, tag=f"w{c}",
                                   name=f"w{c}_{q}_t")
                    nc.sync.dma_start(t[:, :], wd[(c, q)].ap())
                    wtiles[(c, q)] = t
                return wtiles[(c, q)]

            chs = []
            for c in range(NCH):
                S = {}
                S["X"] = [cpool.tile([120, NL], BF16, tag=f"XA{c}", name=f"XA{c}"),
                          cpool.tile([120, NL], BF16, tag=f"XB{c}", name=f"XB{c}")]
                S["G"] = [cpool.tile([48, NL], F32, tag=f"GA{c}", name=f"GA{c}"),
                          cpool.tile([48, NL], F32, tag=f"GB{c}", name=f"GB{c}")]
                S["RW"] = cpool.tile([48, NL], F32, tag=f"RW{c}", name=f"RW{c}")
                S["SG"] = cpool.tile([128, NL], BF16, tag=f"SG{c}", name=f"SG{c}")
                S["SP"] = cpool.tile([128, NL], BF16, tag=f"SP{c}", name=f"SP{c}")
                S["MOB"] = cpool.tile([32, 1], F32, tag=f"MOB{c}", name=f"MOB{c}")
                S["ACC"] = pa.tile([8, NL], F32, tag=f"ACC{c}", name=f"ACC{c}")
                S["tagD"] = f"Dp{c}"
                S["tagV"] = f"VDp{c}"
                S["tagP"] = f"PHp{c}"
                S["tagPP"] = f"PP{c}"
                S["tagM"] = f"M1{c}"
                chs.append(S)

            Z = cpool.tile([32, NL], BF16, tag="Z", name="Z")
            nc.vector.memset(Z[:, :], 0.0)

            for c, S in enumerate(chs):
                nc.sync.dma_start(S["X"][0][:, :], xa_d.ap())
                nc.sync.dma_start(S["X"][1][:, :], xb_d.ap())
                nc.sync.dma_start(S["G"][0][:, :], gi_d.ap())
                nc.vector.memset(S["SG"][:, :], 0.0)
                nc.gpsimd.memset(S["SP"][:, :], 0.0)
                nc.vector.memset(S["MOB"][:, :], Mo_f)
                nc.tensor.matmul(S["ACC"][:, :], wt["WAC1D"][:, :], Z[:, :],
                                 start=True, stop=False)
                w0 = get_wtile(c, 0)
                nc.vector.tensor_add(S["RW"][:, :], S["G"][0][:, :],
                                     w0[:, 0:NL])
                get_wtile(c, 1)

            def step_ops(S, p, wv_next, b, do_gate, do_cost, do_acc):
                """One step of one chain; p = input parity, b = SG block."""
                Xi, Xo = S["X"][p], S["X"][1 - p]
                Go = S["G"][1 - p]
                RW, SG, SP = S["RW"], S["SG"], S["SP"]
                Dp = pp.tile([48, NL], F32, tag=S["tagD"])
                ops = []
                ops.append(lambda: nc.scalar.activation(
                    Xi[32:64, :], Xi[0:32, :], AF.Tanh))
                ops.append(lambda: nc.scalar.activation(
                    Xi[64:96, :], Xi[32:64, :], AF.Square))
                ops.append(lambda: nc.tensor.matmul(
                    Dp[:, :], wt["WX"][:, :], Xi[:, :],
                    start=True, stop=True))
                ops.append(lambda: nc.vector.tensor_add(
                    Xo[0:32, :], Dp[0:32, :], RW[0:32, :]))
                ops.append(lambda: nc.vector.tensor_add(
                    Go[:, :], Dp[:, :], RW[:, :]))
                if do_gate:
                    VDp = pp.tile([80, NL], F32, tag=S["tagV"])
                    PHp = pp.tile([32, NL], F32, tag=S["tagP"])
                    PP = sp.tile([32, NL], BF16, tag=S["tagPP"])
                    M1 = sp.tile([16, NL], F32, tag=S["tagM"])
                    ops.append(lambda: nc.gpsimd.tensor_copy(
                        Xo[96:112, :], Go[32:48, :]))
                    ops.append(lambda: nc.tensor.matmul(
                        VDp[:, :], wt["WVD"][:, :], Xo[:, :],
                        start=True, stop=True))
                    ops.append(lambda: nc.vector.tensor_mul(
                        PP[:, :], Xo[0:32, :], VDp[0:32, :]))
                    if do_cost:
                        ops.append(lambda: nc.vector.tensor_mul(
                            SP[32 * b:32 * b + 16, :],
                            Xo[96:112, :], VDp[64:80, :]))
                    ops.append(lambda: nc.tensor.matmul(
                        PHp[:, :], wt["WPH"][:, :], PP[:, :],
                        start=True, stop=True))
                    ops.append(lambda: nc.scalar.activation(
                        SG[32 * b:32 * b + 32, :], PHp[:, :], AF.Sigmoid,
                        bias=S["MOB"][:, :]))
                    ops.append(lambda: nc.vector.tensor_mul(
                        M1[:, :], SG[32 * b:32 * b + 16, :], VDp[32:48, :]))
                    ops.append(lambda: nc.gpsimd.tensor_add(
                        Go[0:16, :], Go[0:16, :], M1[:, :]))
                    ops.append(lambda: nc.gpsimd.tensor_add(
                        Xo[0:16, :], Xo[0:16, :], M1[:, :]))
                    if do_acc:
                        ops.append(lambda: nc.tensor.matmul(
                            S["ACC"][:, :], wt["WAC4D"][:, :], SG[:, :],
                            start=False, stop=False))
                        ops.append(lambda: nc.tensor.matmul(
                            S["ACC"][:, :], wt["WAC4P"][:, :], SP[:, :],
                            start=False, stop=False))
                if wv_next is not None:
                    ops.append(lambda: nc.gpsimd.tensor_add(
                        RW[:, :], Go[:, :], wv_next))
                return ops

            def emit_group(k0, wts, wts_next_chunk):
                """4 steps (k0..k0+3) for both chains, op-interleaved."""
                for j in range(4):
                    k = k0 + j
                    do_gate = k <= TR - 2
                    do_cost = k <= TR - 3
                    do_acc = do_cost and (j == 3)
                    opsl = []
                    for c, S in enumerate(chs):
                        wtile, base = wts[c]
                        if k + 1 <= TR - 1:
                            if j == 3 and wts_next_chunk is not None:
                                nwtile, nbase = wts_next_chunk[c]
                                wv_next = nwtile[:, nbase:nbase + NL]
                            elif isinstance(base, int):
                                nb = base + (j + 1) * NL
                                wv_next = wtile[:, nb:nb + NL]
                            else:
                                wv_next = wtile[:, bass.ds(base + (j + 1) * NL, NL)]
                        else:
                            wv_next = None
                        opsl.append(step_ops(S, j % 2, wv_next, j,
                                             do_gate, do_cost, do_acc))
                    sa, sb = opsl
                    off = 7   # stagger chain B ~half a step behind A
                    n = max(len(sa), len(sb))
                    for i in range(n + off):
                        if i < len(sa):
                            sa[i]()
                        if 0 <= i - off < len(sb):
                            sb[i - off]()

            # chunks of 256 steps: 63 hw-loop groups + 4 static tail steps
            for q in range(NQ):
                for c in range(NCH):
                    get_wtile(c, q)
                with tc.For_i(0, 63, 1) as iv:
                    wts = [(wtiles[(c, q)], iv * (4 * NL)) for c in range(NCH)]
                    emit_group(q * CH, wts, None)
                k0 = q * CH + 252
                nxt = None
                if q + 1 < NQ:
                    nxt = [(get_wtile(c, q + 1), 0) for c in range(NCH)]
                wts = [(wtiles[(c, q)], 252 * NL) for c in range(NCH)]
                emit_group(k0, wts, nxt)

            # epilogue: after 2047 steps state parity lands in G[1]
            for c, S in enumerate(chs):
                Gl = S["G"][1]
                FSQ = sp.tile([16, NL], BF16, tag=f"FSQ{c}")
                nc.vector.tensor_mul(FSQ[:, :], Gl[32:48, :], Gl[32:48, :])
                nc.tensor.matmul(S["ACC"][:, :], wt["WFIN"][:, :], FSQ[:, :],
                                 start=False, stop=False)
                nc.tensor.matmul(S["ACC"][:, :], wt["WAC1D"][:, :],
                                 S["SG"][0:32, :], start=False, stop=False)
                nc.tensor.matmul(S["ACC"][:, :], wt["WAC1P"][:, :],
                                 S["SP"][0:32, :], start=False, stop=True)
                OUT = sp.tile([8, NL], F32, tag=f"OUT{c}")
                nc.scalar.activation(OUT[:, :], S["ACC"][:, :], AF.Copy,
                                     bias=float(init_c))
                nc.sync.dma_start(out_d[c].ap(), OUT[:, :])
    ctx.close()
    nc.compile()
    return nc


def _pack_w(w_core):
    """w_core [512, 2, 2047] f32 -> {(c,q): [48, 256*32] f32}."""
    out = {}
    T2 = NQ * CH
    for c in range(NCH):
        wc = w_core[c * LCH:(c + 1) * LCH].reshape(NG, NL, 2, TR)
        arr = np.zeros((48, T2, NL), np.float32)
        for g in range(NG):
            arr[16 + g, :TR, :] = wc[g, :, 0, :].T
            arr[24 + g, :TR, :] = wc[g, :, 1, :].T
            arr[32 + g, :TR, :] = wc[g, :, 0, :].T
            arr[40 + g, :TR, :] = wc[g, :, 1, :].T
        for q in range(NQ):
            out[(c, q)] = np.ascontiguousarray(
                arr[:, q * CH:(q + 1) * CH, :]).reshape(48, CH * NL)
    return out


_PROG_CACHE = {}


def kernel(w, K, L, M, Mo):
    w = np.asarray(w, np.float32)
    K = np.asarray(K, np.float32)
    L = np.asarray(L, np.float32)
    M = np.asarray(M, np.float32)
    Mo = np.asarray(Mo, np.float32)
    B = w.shape[0]
    Wmats, Mo_f, init_c = _build_weights(K, L, M, Mo)

    key = (w.shape, K.tobytes(), L.tobytes(), M.tobytes(), Mo.tobytes())
    if key not in _PROG_CACHE:
        _PROG_CACHE[key] = _build_program(Mo_f, init_c)
    nc = _PROG_CACHE[key]

    xa, xb, gi = _init_consts()
    in_maps = []
    for core in range(N_CORES):
        m = {n: np.asarray(Wmats[n]) for n in Wmats}
        m["XIA"], m["XIB"], m["GI"] = xa, xb, gi
        wp = _pack_w(w[core * LPC:(core + 1) * LPC])
        for (c, q), arr in wp.items():
            m[f"w{c}_{q}"] = arr
        in_maps.append(m)

    kw = {}
    if os.environ.get("KERNEL_TRACE"):
        kw = dict(trace=True)
        if os.environ.get("KERNEL_TRACE_DIR"):
            kw["tmpdir"] = os.environ["KERNEL_TRACE_DIR"]
    res = bass_utils.run_bass_kernel_spmd(nc, in_maps,
                                          core_ids=list(range(N_CORES)), **kw)
    globals()["_LAST_RES"] = res
    out = np.empty(B, np.float32)
    for core in range(N_CORES):
        for c in range(NCH):
            o = res.results[core][f"out{c}"]       # [8, 32]
            lo = core * LPC + c * LCH
            out[lo:lo + LCH] = o.reshape(LCH)
    return out


# revision 13
# speedup vs baseline: 1.6145x; 1.0801x over previous
"""Trainium2 Bass kernel for nn_CSTR: B=4096-lane vmapped 2047-step rollout.

v3: data-parallel over 8 cores (512 lanes each), 2 independent 256-lane
chains per core, emission-staggered half a step apart so their
instruction streams fill each other's dependency gaps.

Per chain: slot-major layout, slot = 8 partitions x 32 free lanes,
state slots (x1, x2, xh1, xh2). Mega input tile X[104,32] bf16 =
[state@0; tanh@32; tanh^2@64; ones@96] (every engine-written sub-view
32-partition aligned). The full state update collapses to ONE bf16
matmul producing the O(H) increment Delta (u = K@xhat composed in, RK4
constants on the ones slot); the exact fp32 state G[32,32] is kept via
a vector add G' = Delta + (G + w). Gating: W_VD -> [Ls@rx + M;
0pad+(x-fp); Qc@x], products on DVE, W_PH -> phi (x4 dup), sigmoid on
ScalarE; the blend add uses a zero-padded delta*(x-fp) [32,32] so both
the fp32 state and bf16 mirror update with one aligned add each.
Stage cost accumulates into a persistent PSUM bank every 4 steps from
two [128,32] staging tiles (deltas, Qc products). Only O(H)-scaled
terms flow through bf16 matmuls; fp32 trajectory exactness is
preserved via the vector adds (2.7e-4 max rel err vs reference).
"""
import os
import sys
import numpy as np
import ml_dtypes
from contextlib import ExitStack

sys.path.insert(0, "/opt/trn_rl_repo")

import concourse.bacc as bacc
import concourse.bass as bass
import concourse.mybir as mybir
import concourse.tile as tile
from concourse import bass_utils

F32 = mybir.dt.float32
BF16 = mybir.dt.bfloat16
AF = mybir.ActivationFunctionType

H = np.float32(0.01)
LAM = np.float32(1.0)
B_TOT, N_CORES = 4096, 8
LPC = 512                 # lanes per core
NCH = 2                   # chains per core
LCH = 256                 # lanes per chain
NG, NL = 8, 32            # groups x free lanes per chain
TR = 2047                 # real steps
CH = 256                  # steps per w chunk
NQ = 8                    # chunks (last holds 255 real steps)

C1 = np.float32(1.0) - H
GC = np.float32(0.5) * H
EC = np.float32(H * H / 2)
A0 = np.float32(5e-5)
ALPHA = np.float32(H - np.float32(1e-6) / 3)


def _kron8(A):
    """lhsT for slot map A[out_slot, in_slot] -> [8*in, 8*out] bf16."""
    k = np.kron(np.ascontiguousarray(A.T), np.eye(NG, dtype=np.float32))
    return k.astype(ml_dtypes.bfloat16)


def _build_weights(K, L, M, Mo):
    K1, K2 = np.float32(K[0, 0]), np.float32(K[0, 1])
    Ls = ((L + L.T) * np.float32(0.5)).astype(np.float32)
    Mv = M[0].astype(np.float32)
    Qc = np.array([[1 + K1 * K1, K1 * K2], [K1 * K2, 1 + K2 * K2]], np.float32)
    # slot order (x1, x2, xh1, xh2) == rx order (x1, x2, fp1, fp2)

    # W_X inputs: state(4) T(4) Q(4) ones(1) = 13 slots -> Delta(4)
    uc = np.array([H, GC, H, GC], np.float32)
    Kv = np.array([0, 0, K1, K2], np.float32)
    A_S = (C1 - 1) * np.eye(4, dtype=np.float32) + np.outer(uc, Kv)
    A_T = np.zeros((4, 4), np.float32)
    A_T[0, 1] = ALPHA; A_T[1, 0] = -H; A_T[2, 3] = ALPHA; A_T[3, 2] = -H
    A_Q = np.zeros((4, 4), np.float32)
    A_Q[0, 1] = -A0; A_Q[2, 3] = -A0
    A_ONES = np.array([A0, -EC, A0, -EC], np.float32).reshape(4, 1)
    WX = np.concatenate([A_S, A_T, A_Q, A_ONES], axis=1)   # [4, 13]

    # W_VD: 13 input slots -> [y(4); 0pad(2); DIF(2); y2(2)] = 10 out
    Avd = np.zeros((10, 13), np.float32)
    for s in range(4):
        for sp in range(4):
            Avd[s, sp] = Ls[s, sp]
        Avd[s, 12] = Mv[s]
    Avd[6, 0] = 1; Avd[6, 2] = -1          # DIF1 = x1 - xh1
    Avd[7, 1] = 1; Avd[7, 3] = -1          # DIF2 = x2 - xh2
    Avd[8, 0] = Qc[0, 0]; Avd[8, 1] = Qc[0, 1]
    Avd[9, 0] = Qc[1, 0]; Avd[9, 1] = Qc[1, 1]

    APH = np.ones((4, 4), np.float32)      # phi, x4 dup
    AC4D = np.zeros((1, 16), np.float32)
    AC4P = np.zeros((1, 16), np.float32)
    for b in range(4):
        AC4D[0, 4 * b] = LAM
        AC4P[0, 4 * b] = 1; AC4P[0, 4 * b + 1] = 1
    AC1D = np.zeros((1, 4), np.float32); AC1D[0, 0] = LAM
    AC1P = np.zeros((1, 4), np.float32); AC1P[0, 0] = 1; AC1P[0, 1] = 1
    AFIN = np.array([[10.0, 10.0]], np.float32)

    W = {
        "WX": _kron8(WX),        # [104, 32]
        "WVD": _kron8(Avd),      # [104, 80]
        "WPH": _kron8(APH),      # [32, 32]
        "WAC4D": _kron8(AC4D),   # [128, 8]
        "WAC4P": _kron8(AC4P),   # [128, 8]
        "WAC1D": _kron8(AC1D),   # [32, 8]
        "WAC1P": _kron8(AC1P),   # [32, 8]
        "WFIN": _kron8(AFIN),    # [16, 8]
    }
    init_c = float(1.0 + K1 * K1 + LAM)
    return W, float(Mo[0, 0]), init_c


WSHAPES = {"WX": (104, 32), "WVD": (104, 80), "WPH": (32, 32),
           "WAC4D": (128, 8), "WAC4P": (128, 8), "WAC1D": (32, 8),
           "WAC1P": (32, 8), "WFIN": (16, 8)}


def _init_consts():
    xa = np.zeros((104, NL), np.float32)
    for lo, v in ((0, 1.0), (8, 0.0), (16, 1.0), (24, 0.0)):   # state
        xa[lo:lo + 8] = v
    xa[96:104] = 1.0                                           # ones
    xb = np.zeros((104, NL), np.float32)
    xb[96:104] = 1.0
    gi = np.zeros((32, NL), np.float32)
    gi[0:8] = 1.0; gi[16:24] = 1.0
    return (xa.astype(ml_dtypes.bfloat16), xb.astype(ml_dtypes.bfloat16), gi)


def _build_program(Mo_f, init_c):
    nc = bacc.Bacc("TRN2", target_bir_lowering=False, debug=False)

    wd = {}
    for c in range(NCH):
        for q in range(NQ):
            wd[(c, q)] = nc.dram_tensor(
                f"w{c}_{q}", [32, CH * NL], F32, kind="ExternalInput")
    wdram = {n: nc.dram_tensor(n, list(WSHAPES[n]), BF16, kind="ExternalInput")
             for n in WSHAPES}
    xa_d = nc.dram_tensor("XIA", [104, NL], BF16, kind="ExternalInput")
    xb_d = nc.dram_tensor("XIB", [104, NL], BF16, kind="ExternalInput")
    gi_d = nc.dram_tensor("GI", [32, NL], F32, kind="ExternalInput")
    out_d = [nc.dram_tensor(f"out{c}", [NG, NL], F32, kind="ExternalOutput")
             for c in range(NCH)]

    ctx = ExitStack()
    with tile.TileContext(nc) as tc:
        with tc.tile_pool(name="wq", bufs=2) as wpool, \
             tc.tile_pool(name="wt", bufs=1) as cpool, \
             tc.tile_pool(name="sb", bufs=3) as sp, \
             tc.tile_pool(name="ps", bufs=1, space="PSUM") as pp, \
             tc.tile_pool(name="pacc", bufs=1, space="PSUM") as pa:

            wt = {}
            for n in WSHAPES:
                wt[n] = cpool.tile(list(WSHAPES[n]), BF16, tag=n, name=n + "_t")
                nc.sync.dma_start(wt[n][:, :], wdram[n].ap())

            wtiles = {}

            def get_wtile(c, q):
                if (c, q) not in wtiles:
                    t = wpool.tile([32, CH * NL], F32, tag=f"w{c}",
                                   name=f"w{c}_{q}_t")
                    nc.sync.dma_start(t[:, :], wd[(c, q)].ap())
                    wtiles[(c, q)] = t
                return wtiles[(c, q)]

            chs = []
            for c in range(NCH):
                S = {}
                S["X"] = [cpool.tile([104, NL], BF16, tag=f"XA{c}", name=f"XA{c}"),
                          cpool.tile([104, NL], BF16, tag=f"XB{c}", name=f"XB{c}")]
                S["G"] = [cpool.tile([32, NL], F32, tag=f"GA{c}", name=f"GA{c}"),
                          cpool.tile([32, NL], F32, tag=f"GB{c}", name=f"GB{c}")]
                S["RW"] = cpool.tile([32, NL], F32, tag=f"RW{c}", name=f"RW{c}")
                S["SG"] = cpool.tile([128, NL], BF16, tag=f"SG{c}", name=f"SG{c}")
                S["SP"] = cpool.tile([128, NL], BF16, tag=f"SP{c}", name=f"SP{c}")
                S["MOB"] = cpool.tile([32, 1], F32, tag=f"MOB{c}", name=f"MOB{c}")
                S["ACC"] = pa.tile([8, NL], F32, tag=f"ACC{c}", name=f"ACC{c}")
                S["tagD"] = f"Dp{c}"
                S["tagV"] = f"VDp{c}"
                S["tagP"] = f"PHp{c}"
                S["tagPP"] = f"PP{c}"
                S["tagM"] = f"M1{c}"
                chs.append(S)

            Z = cpool.tile([32, NL], BF16, tag="Z", name="Z")
            nc.vector.memset(Z[:, :], 0.0)

            for c, S in enumerate(chs):
                nc.sync.dma_start(S["X"][0][:, :], xa_d.ap())
                nc.sync.dma_start(S["X"][1][:, :], xb_d.ap())
                nc.sync.dma_start(S["G"][0][:, :], gi_d.ap())
                nc.vector.memset(S["SG"][:, :], 0.0)
                nc.gpsimd.memset(S["SP"][:, :], 0.0)
                nc.vector.memset(S["MOB"][:, :], Mo_f)
                nc.tensor.matmul(S["ACC"][:, :], wt["WAC1D"][:, :], Z[:, :],
                                 start=True, stop=False)
                w0 = get_wtile(c, 0)
                nc.vector.tensor_add(S["RW"][:, :], S["G"][0][:, :],
                                     w0[:, 0:NL])
                get_wtile(c, 1)

            def step_ops(S, p, wv_next, b, do_gate, do_cost, do_acc):
                """One step of one chain; p = input parity, b = SG block."""
                Xi, Xo = S["X"][p], S["X"][1 - p]
                Go = S["G"][1 - p]
                RW, SG, SP = S["RW"], S["SG"], S["SP"]
                Dp = pp.tile([32, NL], F32, tag=S["tagD"])
                ops = []
                ops.append(lambda: nc.scalar.activation(
                    Xi[32:64, :], Xi[0:32, :], AF.Tanh))
                ops.append(lambda: nc.scalar.activation(
                    Xi[64:96, :], Xi[32:64, :], AF.Square))
                ops.append(lambda: nc.tensor.matmul(
                    Dp[:, :], wt["WX"][:, :], Xi[:, :],
                    start=True, stop=True))
                ops.append(lambda: nc.vector.tensor_add(
                    Xo[0:32, :], Dp[:, :], RW[:, :]))
                ops.append(lambda: nc.vector.tensor_add(
                    Go[:, :], Dp[:, :], RW[:, :]))
                if do_gate:
                    VDp = pp.tile([80, NL], F32, tag=S["tagV"])
                    PHp = pp.tile([32, NL], F32, tag=S["tagP"])
                    PP = sp.tile([32, NL], BF16, tag=S["tagPP"])
                    M1 = sp.tile([32, NL], F32, tag=S["tagM"])
                    ops.append(lambda: nc.tensor.matmul(
                        VDp[:, :], wt["WVD"][:, :], Xo[:, :],
                        start=True, stop=True))
                    ops.append(lambda: nc.vector.tensor_mul(
                        PP[:, :], Xo[0:32, :], VDp[0:32, :]))
                    if do_cost:
                        ops.append(lambda: nc.vector.tensor_mul(
                            SP[32 * b:32 * b + 16, :],
                            Xo[0:16, :], VDp[64:80, :]))
                    ops.append(lambda: nc.tensor.matmul(
                        PHp[:, :], wt["WPH"][:, :], PP[:, :],
                        start=True, stop=True))
                    ops.append(lambda: nc.scalar.activation(
                        SG[32 * b:32 * b + 32, :], PHp[:, :], AF.Sigmoid,
                        bias=S["MOB"][:, :]))
                    ops.append(lambda: nc.vector.tensor_mul(
                        M1[:, :], SG[32 * b:32 * b + 32, :], VDp[32:64, :]))
                    ops.append(lambda: nc.gpsimd.tensor_add(
                        Go[:, :], Go[:, :], M1[:, :]))
                    ops.append(lambda: nc.gpsimd.tensor_add(
                        Xo[0:32, :], Xo[0:32, :], M1[:, :]))
                    if do_acc:
                        ops.append(lambda: nc.tensor.matmul(
                            S["ACC"][:, :], wt["WAC4D"][:, :], SG[:, :],
                            start=False, stop=False))
                        ops.append(lambda: nc.tensor.matmul(
                            S["ACC"][:, :], wt["WAC4P"][:, :], SP[:, :],
                            start=False, stop=False))
                if wv_next is not None:
                    ops.append(lambda: nc.gpsimd.tensor_add(
                        RW[:, :], Go[:, :], wv_next))
                return ops

            def emit_group(k0, wts, wts_next_chunk):
                """4 steps (k0..k0+3) for both chains, B staggered."""
                for j in range(4):
                    k = k0 + j
                    do_gate = k <= TR - 2
                    do_cost = k <= TR - 3
                    do_acc = do_cost and (j == 3)
                    opsl = []
                    for c, S in enumerate(chs):
                        wtile, base = wts[c]
                        if k + 1 <= TR - 1:
                            if j == 3 and wts_next_chunk is not None:
                                nwtile, nbase = wts_next_chunk[c]
                                wv_next = nwtile[:, nbase:nbase + NL]
                            elif isinstance(base, int):
                                nb = base + (j + 1) * NL
                                wv_next = wtile[:, nb:nb + NL]
                            else:
                                wv_next = wtile[:, bass.ds(base + (j + 1) * NL, NL)]
                        else:
                            wv_next = None
                        opsl.append(step_ops(S, j % 2, wv_next, j,
                                             do_gate, do_cost, do_acc))
                    sa, sb = opsl
                    off = 6   # stagger chain B ~half a step behind A
                    for i in range(max(len(sa), len(sb)) + off):
                        if i < len(sa):
                            sa[i]()
                        if 0 <= i - off < len(sb):
                            sb[i - off]()

            # chunks of 256 steps: 63 hw-loop groups + 4 static tail steps
            for q in range(NQ):
                for c in range(NCH):
                    get_wtile(c, q)
                with tc.For_i(0, 63, 1) as iv:
                    wts = [(wtiles[(c, q)], iv * (4 * NL)) for c in range(NCH)]
                    emit_group(q * CH, wts, None)
                k0 = q * CH + 252
                nxt = None
                if q + 1 < NQ:
                    nxt = [(get_wtile(c, q + 1), 0) for c in range(NCH)]
                wts = [(wtiles[(c, q)], 252 * NL) for c in range(NCH)]
                emit_group(k0, wts, nxt)

            # epilogue: after 2047 steps state parity lands in G[1]
            for c, S in enumerate(chs):
                Gl = S["G"][1]
                FSQ = sp.tile([16, NL], BF16, tag=f"FSQ{c}")
                nc.vector.tensor_mul(FSQ[:, :], Gl[0:16, :], Gl[0:16, :])
                nc.tensor.matmul(S["ACC"][:, :], wt["WFIN"][:, :], FSQ[:, :],
                                 start=False, stop=False)
                nc.tensor.matmul(S["ACC"][:, :], wt["WAC1D"][:, :],
                                 S["SG"][0:32, :], start=False, stop=False)
                nc.tensor.matmul(S["ACC"][:, :], wt["WAC1P"][:, :],
                                 S["SP"][0:32, :], start=False, stop=True)
                OUT = sp.tile([8, NL], F32, tag=f"OUT{c}")
                nc.scalar.activation(OUT[:, :], S["ACC"][:, :], AF.Copy,
                                     bias=float(init_c))
                nc.sync.dma_start(out_d[c].ap(), OUT[:, :])
    ctx.close()
    nc.compile()
    return nc


def _pack_w(w_core):
    """w_core [512, 2, 2047] f32 -> {(c,q): [32, 256*32] f32}."""
    out = {}
    T2 = NQ * CH
    for c in range(NCH):
        wc = w_core[c * LCH:(c + 1) * LCH].reshape(NG, NL, 2, TR)
        arr = np.zeros((32, T2, NL), np.float32)
        for g in range(NG):
            arr[g, :TR, :] = wc[g, :, 0, :].T         # x1 slot
            arr[8 + g, :TR, :] = wc[g, :, 1, :].T     # x2 slot
        for q in range(NQ):
            out[(c, q)] = np.ascontiguousarray(
                arr[:, q * CH:(q + 1) * CH, :]).reshape(32, CH * NL)
    return out


_PROG_CACHE = {}


def kernel(w, K, L, M, Mo):
    w = np.asarray(w, np.float32)
    K = np.asarray(K, np.float32)
    L = np.asarray(L, np.float32)
    M = np.asarray(M, np.float32)
    Mo = np.asarray(Mo, np.float32)
    B = w.shape[0]
    Wmats, Mo_f, init_c = _build_weights(K, L, M, Mo)

    key = (w.shape, K.tobytes(), L.tobytes(), M.tobytes(), Mo.tobytes())
    if key not in _PROG_CACHE:
        _PROG_CACHE[key] = _build_program(Mo_f, init_c)
    nc = _PROG_CACHE[key]

    xa, xb, gi = _init_consts()
    in_maps = []
    for core in range(N_CORES):
        m = {n: np.asarray(Wmats[n]) for n in Wmats}
        m["XIA"], m["XIB"], m["GI"] = xa, xb, gi
        wp = _pack_w(w[core * LPC:(core + 1) * LPC])
        for (c, q), arr in wp.items():
            m[f"w{c}_{q}"] = arr
        in_maps.append(m)

    kw = {}
    if os.environ.get("KERNEL_TRACE"):
        kw = dict(trace=True)
        if os.environ.get("KERNEL_TRACE_DIR"):
            kw["tmpdir"] = os.environ["KERNEL_TRACE_DIR"]
    res = bass_utils.run_bass_kernel_spmd(nc, in_maps,
                                          core_ids=list(range(N_CORES)), **kw)
    globals()["_LAST_RES"] = res
    out = np.empty(B, np.float32)
    for core in range(N_CORES):
        for c in range(NCH):
            o = res.results[core][f"out{c}"]       # [8, 32]
            lo = core * LPC + c * LCH
            out[lo:lo + LCH] = o.reshape(LCH)
    return out


# revision 15
# speedup vs baseline: 1.6850x; 1.0437x over previous
"""Trainium2 Bass kernel for nn_CSTR: B=4096-lane vmapped 2047-step rollout.

v3: data-parallel over 8 cores (512 lanes each), 2 independent 256-lane
chains per core, emission-staggered half a step apart so their
instruction streams fill each other's dependency gaps.

Per chain: slot-major layout, slot = 8 partitions x 32 free lanes,
state slots (x1, x2, xh1, xh2). Mega input tile X[104,32] bf16 =
[state@0; tanh@32; tanh^2@64; ones@96] (every engine-written sub-view
32-partition aligned). The full state update collapses to ONE bf16
matmul producing the O(H) increment Delta (u = K@xhat composed in, RK4
constants on the ones slot); the exact fp32 state G[32,32] is kept via
a vector add G' = Delta + (G + w). Gating: W_VD -> [Ls@rx + M;
0pad+(x-fp); Qc@x], products on DVE, W_PH -> phi (x4 dup), sigmoid on
ScalarE; the blend add uses a zero-padded delta*(x-fp) [32,32] so both
the fp32 state and bf16 mirror update with one aligned add each.
Stage cost accumulates into a persistent PSUM bank every 4 steps from
two [128,32] staging tiles (deltas, Qc products). Only O(H)-scaled
terms flow through bf16 matmuls; fp32 trajectory exactness is
preserved via the vector adds (2.7e-4 max rel err vs reference).
"""
import os
import sys
import numpy as np
import ml_dtypes
from contextlib import ExitStack

sys.path.insert(0, "/opt/trn_rl_repo")

import concourse.bacc as bacc
import concourse.bass as bass
import concourse.mybir as mybir
import concourse.tile as tile
from concourse import bass_utils

F32 = mybir.dt.float32
BF16 = mybir.dt.bfloat16
AF = mybir.ActivationFunctionType

H = np.float32(0.01)
LAM = np.float32(1.0)
B_TOT, N_CORES = 4096, 8
LPC = 512                 # lanes per core
NCH = 2                   # chains per core
LCH = 256                 # lanes per chain
NG, NL = 8, 32            # groups x free lanes per chain
TR = 2047                 # real steps
CH = 256                  # steps per w chunk
NQ = 8                    # chunks (last holds 255 real steps)

C1 = np.float32(1.0) - H
GC = np.float32(0.5) * H
EC = np.float32(H * H / 2)
A0 = np.float32(5e-5)
ALPHA = np.float32(H - np.float32(1e-6) / 3)


def _kron8(A):
    """lhsT for slot map A[out_slot, in_slot] -> [8*in, 8*out] bf16."""
    k = np.kron(np.ascontiguousarray(A.T), np.eye(NG, dtype=np.float32))
    return k.astype(ml_dtypes.bfloat16)


def _build_weights(K, L, M, Mo):
    K1, K2 = np.float32(K[0, 0]), np.float32(K[0, 1])
    Ls = ((L + L.T) * np.float32(0.5)).astype(np.float32)
    Mv = M[0].astype(np.float32)
    Qc = np.array([[1 + K1 * K1, K1 * K2], [K1 * K2, 1 + K2 * K2]], np.float32)
    # slot order (x1, x2, xh1, xh2) == rx order (x1, x2, fp1, fp2)

    # W_X inputs: state(4) T(4) Q(4) ones(1) = 13 slots -> Delta(4)
    uc = np.array([H, GC, H, GC], np.float32)
    Kv = np.array([0, 0, K1, K2], np.float32)
    A_S = (C1 - 1) * np.eye(4, dtype=np.float32) + np.outer(uc, Kv)
    A_T = np.zeros((4, 4), np.float32)
    A_T[0, 1] = ALPHA; A_T[1, 0] = -H; A_T[2, 3] = ALPHA; A_T[3, 2] = -H
    A_Q = np.zeros((4, 4), np.float32)
    A_Q[0, 1] = -A0; A_Q[2, 3] = -A0
    A_ONES = np.array([A0, -EC, A0, -EC], np.float32).reshape(4, 1)
    WX = np.concatenate([A_S, A_T, A_Q, A_ONES], axis=1)   # [4, 13]

    # W_VD: 13 input slots -> [y(4); 0pad(2); DIF(2); y2(2)] = 10 out
    Avd = np.zeros((10, 13), np.float32)
    for s in range(4):
        for sp in range(4):
            Avd[s, sp] = Ls[s, sp]
        Avd[s, 12] = Mv[s]
    Avd[6, 0] = 1; Avd[6, 2] = -1          # DIF1 = x1 - xh1
    Avd[7, 1] = 1; Avd[7, 3] = -1          # DIF2 = x2 - xh2
    Avd[8, 0] = Qc[0, 0]; Avd[8, 1] = Qc[0, 1]
    Avd[9, 0] = Qc[1, 0]; Avd[9, 1] = Qc[1, 1]

    APH = np.ones((4, 4), np.float32)      # phi, x4 dup
    AC4D = np.zeros((1, 16), np.float32)
    AC4P = np.zeros((1, 16), np.float32)
    for b in range(4):
        AC4D[0, 4 * b] = LAM
        AC4P[0, 4 * b] = 1; AC4P[0, 4 * b + 1] = 1
    AC1D = np.zeros((1, 4), np.float32); AC1D[0, 0] = LAM
    AC1P = np.zeros((1, 4), np.float32); AC1P[0, 0] = 1; AC1P[0, 1] = 1
    AFIN = np.array([[10.0, 10.0]], np.float32)

    W = {
        "WX": _kron8(WX),        # [104, 32]
        "WVD": _kron8(Avd),      # [104, 80]
        "WPH": _kron8(APH),      # [32, 32]
        "WAC4D": _kron8(AC4D),   # [128, 8]
        "WAC4P": _kron8(AC4P),   # [128, 8]
        "WAC1D": _kron8(AC1D),   # [32, 8]
        "WAC1P": _kron8(AC1P),   # [32, 8]
        "WFIN": _kron8(AFIN),    # [16, 8]
    }
    init_c = float(1.0 + K1 * K1 + LAM)
    return W, float(Mo[0, 0]), init_c


WSHAPES = {"WX": (104, 32), "WVD": (104, 80), "WPH": (32, 32),
           "WAC4D": (128, 8), "WAC4P": (128, 8), "WAC1D": (32, 8),
           "WAC1P": (32, 8), "WFIN": (16, 8)}


def _init_consts():
    xa = np.zeros((104, NL), np.float32)
    for lo, v in ((0, 1.0), (8, 0.0), (16, 1.0), (24, 0.0)):   # state
        xa[lo:lo + 8] = v
    xa[96:104] = 1.0                                           # ones
    xb = np.zeros((104, NL), np.float32)
    xb[96:104] = 1.0
    gi = np.zeros((32, NL), np.float32)
    gi[0:8] = 1.0; gi[16:24] = 1.0
    return (xa.astype(ml_dtypes.bfloat16), xb.astype(ml_dtypes.bfloat16), gi)


def _build_program(Mo_f, init_c):
    nc = bacc.Bacc("TRN2", target_bir_lowering=False, debug=False)

    wd = {}
    for c in range(NCH):
        for q in range(NQ):
            wd[(c, q)] = nc.dram_tensor(
                f"w{c}_{q}", [32, CH * NL], F32, kind="ExternalInput")
    wdram = {n: nc.dram_tensor(n, list(WSHAPES[n]), BF16, kind="ExternalInput")
             for n in WSHAPES}
    xa_d = nc.dram_tensor("XIA", [104, NL], BF16, kind="ExternalInput")
    xb_d = nc.dram_tensor("XIB", [104, NL], BF16, kind="ExternalInput")
    gi_d = nc.dram_tensor("GI", [32, NL], F32, kind="ExternalInput")
    out_d = [nc.dram_tensor(f"out{c}", [NG, NL], F32, kind="ExternalOutput")
             for c in range(NCH)]

    ctx = ExitStack()
    with tile.TileContext(nc) as tc:
        with tc.tile_pool(name="wq", bufs=2) as wpool, \
             tc.tile_pool(name="wt", bufs=1) as cpool, \
             tc.tile_pool(name="sb", bufs=3) as sp, \
             tc.tile_pool(name="ps", bufs=1, space="PSUM") as pp, \
             tc.tile_pool(name="pacc", bufs=1, space="PSUM") as pa:

            wt = {}
            for n in WSHAPES:
                wt[n] = cpool.tile(list(WSHAPES[n]), BF16, tag=n, name=n + "_t")
                nc.sync.dma_start(wt[n][:, :], wdram[n].ap())

            wtiles = {}

            def get_wtile(c, q):
                if (c, q) not in wtiles:
                    t = wpool.tile([32, CH * NL], F32, tag=f"w{c}",
                                   name=f"w{c}_{q}_t")
                    nc.sync.dma_start(t[:, :], wd[(c, q)].ap())
                    wtiles[(c, q)] = t
                return wtiles[(c, q)]

            chs = []
            for c in range(NCH):
                S = {}
                S["X"] = [cpool.tile([104, NL], BF16, tag=f"XA{c}", name=f"XA{c}"),
                          cpool.tile([104, NL], BF16, tag=f"XB{c}", name=f"XB{c}")]
                S["G"] = [cpool.tile([32, NL], F32, tag=f"GA{c}", name=f"GA{c}"),
                          cpool.tile([32, NL], F32, tag=f"GB{c}", name=f"GB{c}")]
                S["RW"] = cpool.tile([32, NL], F32, tag=f"RW{c}", name=f"RW{c}")
                S["SG"] = cpool.tile([128, NL], BF16, tag=f"SG{c}", name=f"SG{c}")
                S["SP"] = cpool.tile([128, NL], BF16, tag=f"SP{c}", name=f"SP{c}")
                S["MOB"] = cpool.tile([32, 1], F32, tag=f"MOB{c}", name=f"MOB{c}")
                S["ACC"] = pa.tile([8, NL], F32, tag=f"ACC{c}", name=f"ACC{c}")
                S["tagD"] = f"Dp{c}"
                S["tagV"] = f"VDp{c}"
                S["tagP"] = f"PHp{c}"
                S["tagPP"] = f"PP{c}"
                S["tagM"] = f"M1{c}"
                chs.append(S)

            Z = cpool.tile([32, NL], BF16, tag="Z", name="Z")
            nc.vector.memset(Z[:, :], 0.0)

            for c, S in enumerate(chs):
                nc.sync.dma_start(S["X"][0][:, :], xa_d.ap())
                nc.sync.dma_start(S["X"][1][:, :], xb_d.ap())
                nc.sync.dma_start(S["G"][0][:, :], gi_d.ap())
                nc.vector.memset(S["SG"][:, :], 0.0)
                nc.gpsimd.memset(S["SP"][:, :], 0.0)
                nc.vector.memset(S["MOB"][:, :], Mo_f)
                nc.tensor.matmul(S["ACC"][:, :], wt["WAC1D"][:, :], Z[:, :],
                                 start=True, stop=False)
                w0 = get_wtile(c, 0)
                nc.vector.tensor_add(S["RW"][:, :], S["G"][0][:, :],
                                     w0[:, 0:NL])
                get_wtile(c, 1)

            def step_ops(S, p, wv_next, b, do_gate, do_cost, do_acc):
                """One step of one chain; p = input parity, b = SG block."""
                Xi, Xo = S["X"][p], S["X"][1 - p]
                Go = S["G"][1 - p]
                RW, SG, SP = S["RW"], S["SG"], S["SP"]
                Dp = pp.tile([32, NL], F32, tag=S["tagD"])
                ops = []
                ops.append(lambda: nc.scalar.activation(
                    Xi[32:64, :], Xi[0:32, :], AF.Tanh))
                ops.append(lambda: nc.scalar.activation(
                    Xi[64:96, :], Xi[32:64, :], AF.Square))
                ops.append(lambda: nc.tensor.matmul(
                    Dp[:, :], wt["WX"][:, :], Xi[:, :],
                    start=True, stop=True))
                ops.append(lambda: nc.vector.tensor_add(
                    Xo[0:32, :], Dp[:, :], RW[:, :]))
                ops.append(lambda: nc.vector.tensor_add(
                    Go[:, :], Dp[:, :], RW[:, :]))
                if do_gate:
                    VDp = pp.tile([80, NL], F32, tag=S["tagV"])
                    PHp = pp.tile([32, NL], F32, tag=S["tagP"])
                    PP = sp.tile([32, NL], BF16, tag=S["tagPP"])
                    M1 = sp.tile([32, NL], F32, tag=S["tagM"])
                    ops.append(lambda: nc.tensor.matmul(
                        VDp[:, :], wt["WVD"][:, :], Xo[:, :],
                        start=True, stop=True))
                    ops.append(lambda: nc.vector.tensor_mul(
                        PP[:, :], Xo[0:32, :], VDp[0:32, :]))
                    if do_cost:
                        ops.append(lambda: nc.vector.tensor_mul(
                            SP[32 * b:32 * b + 16, :],
                            Xo[0:16, :], VDp[64:80, :]))
                    ops.append(lambda: nc.tensor.matmul(
                        PHp[:, :], wt["WPH"][:, :], PP[:, :],
                        start=True, stop=True))
                    ops.append(lambda: nc.scalar.activation(
                        SG[32 * b:32 * b + 32, :], PHp[:, :], AF.Sigmoid,
                        bias=S["MOB"][:, :]))
                    ops.append(lambda: nc.vector.tensor_mul(
                        M1[:, :], SG[32 * b:32 * b + 32, :], VDp[32:64, :]))
                    ops.append(lambda: nc.vector.tensor_add(
                        Xo[0:32, :], Xo[0:32, :], M1[:, :]))
                    ops.append(lambda: nc.gpsimd.tensor_add(
                        Go[:, :], Go[:, :], M1[:, :]))
                    if do_acc:
                        ops.append(lambda: nc.tensor.matmul(
                            S["ACC"][:, :], wt["WAC4D"][:, :], SG[:, :],
                            start=False, stop=False))
                        ops.append(lambda: nc.tensor.matmul(
                            S["ACC"][:, :], wt["WAC4P"][:, :], SP[:, :],
                            start=False, stop=False))
                if wv_next is not None:
                    ops.append(lambda: nc.gpsimd.tensor_add(
                        RW[:, :], Go[:, :], wv_next))
                return ops

            def emit_group(k0, wts, wts_next_chunk):
                """4 steps (k0..k0+3) for both chains, B staggered."""
                for j in range(4):
                    k = k0 + j
                    do_gate = k <= TR - 2
                    do_cost = k <= TR - 3
                    do_acc = do_cost and (j == 3)
                    opsl = []
                    for c, S in enumerate(chs):
                        wtile, base = wts[c]
                        if k + 1 <= TR - 1:
                            if j == 3 and wts_next_chunk is not None:
                                nwtile, nbase = wts_next_chunk[c]
                                wv_next = nwtile[:, nbase:nbase + NL]
                            elif isinstance(base, int):
                                nb = base + (j + 1) * NL
                                wv_next = wtile[:, nb:nb + NL]
                            else:
                                wv_next = wtile[:, bass.ds(base + (j + 1) * NL, NL)]
                        else:
                            wv_next = None
                        opsl.append(step_ops(S, j % 2, wv_next, j,
                                             do_gate, do_cost, do_acc))
                    sa, sb = opsl
                    off = 8   # stagger chain B ~half a step behind A
                    for i in range(max(len(sa), len(sb)) + off):
                        if i < len(sa):
                            sa[i]()
                        if 0 <= i - off < len(sb):
                            sb[i - off]()

            # chunks of 256 steps: 63 hw-loop groups + 4 static tail steps
            for q in range(NQ):
                for c in range(NCH):
                    get_wtile(c, q)
                with tc.For_i(0, 63, 1) as iv:
                    wts = [(wtiles[(c, q)], iv * (4 * NL)) for c in range(NCH)]
                    emit_group(q * CH, wts, None)
                k0 = q * CH + 252
                nxt = None
                if q + 1 < NQ:
                    nxt = [(get_wtile(c, q + 1), 0) for c in range(NCH)]
                wts = [(wtiles[(c, q)], 252 * NL) for c in range(NCH)]
                emit_group(k0, wts, nxt)

            # epilogue: after 2047 steps state parity lands in G[1]
            for c, S in enumerate(chs):
                Gl = S["G"][1]
                FSQ = sp.tile([16, NL], BF16, tag=f"FSQ{c}")
                nc.vector.tensor_mul(FSQ[:, :], Gl[0:16, :], Gl[0:16, :])
                nc.tensor.matmul(S["ACC"][:, :], wt["WFIN"][:, :], FSQ[:, :],
                                 start=False, stop=False)
                nc.tensor.matmul(S["ACC"][:, :], wt["WAC1D"][:, :],
                                 S["SG"][0:32, :], start=False, stop=False)
                nc.tensor.matmul(S["ACC"][:, :], wt["WAC1P"][:, :],
                                 S["SP"][0:32, :], start=False, stop=True)
                OUT = sp.tile([8, NL], F32, tag=f"OUT{c}")
                nc.scalar.activation(OUT[:, :], S["ACC"][:, :], AF.Copy,
                                     bias=float(init_c))
                nc.sync.dma_start(out_d[c].ap(), OUT[:, :])
    ctx.close()
    nc.compile()
    return nc


def _pack_w(w_core):
    """w_core [512, 2, 2047] f32 -> {(c,q): [32, 256*32] f32}."""
    out = {}
    T2 = NQ * CH
    for c in range(NCH):
        wc = w_core[c * LCH:(c + 1) * LCH].reshape(NG, NL, 2, TR)
        arr = np.zeros((32, T2, NL), np.float32)
        for g in range(NG):
            arr[g, :TR, :] = wc[g, :, 0, :].T         # x1 slot
            arr[8 + g, :TR, :] = wc[g, :, 1, :].T     # x2 slot
        for q in range(NQ):
            out[(c, q)] = np.ascontiguousarray(
                arr[:, q * CH:(q + 1) * CH, :]).reshape(32, CH * NL)
    return out


_PROG_CACHE = {}


def kernel(w, K, L, M, Mo):
    w = np.asarray(w, np.float32)
    K = np.asarray(K, np.float32)
    L = np.asarray(L, np.float32)
    M = np.asarray(M, np.float32)
    Mo = np.asarray(Mo, np.float32)
    B = w.shape[0]
    Wmats, Mo_f, init_c = _build_weights(K, L, M, Mo)

    key = (w.shape, K.tobytes(), L.tobytes(), M.tobytes(), Mo.tobytes())
    if key not in _PROG_CACHE:
        _PROG_CACHE[key] = _build_program(Mo_f, init_c)
    nc = _PROG_CACHE[key]

    xa, xb, gi = _init_consts()
    in_maps = []
    for core in range(N_CORES):
        m = {n: np.asarray(Wmats[n]) for n in Wmats}
        m["XIA"], m["XIB"], m["GI"] = xa, xb, gi
        wp = _pack_w(w[core * LPC:(core + 1) * LPC])
        for (c, q), arr in wp.items():
            m[f"w{c}_{q}"] = arr
        in_maps.append(m)

    kw = {}
    if os.environ.get("KERNEL_TRACE"):
        kw = dict(trace=True)
        if os.environ.get("KERNEL_TRACE_DIR"):
            kw["tmpdir"] = os.environ["KERNEL_TRACE_DIR"]
    res = bass_utils.run_bass_kernel_spmd(nc, in_maps,
                                          core_ids=list(range(N_CORES)), **kw)
    globals()["_LAST_RES"] = res
    out = np.empty(B, np.float32)
    for core in range(N_CORES):
        for c in range(NCH):
            o = res.results[core][f"out{c}"]       # [8, 32]
            lo = core * LPC + c * LCH
            out[lo:lo + LCH] = o.reshape(LCH)
    return out
